# revision 10
# baseline (speedup 1.0000x reference)
"""EntityAttentionLayer on 8 Trainium2 NeuronCores (Bass/Tile).

Reference computation (per batch b of 1024):
    qkv = entities @ W_in.T            # [128 ents, 3*512]
    q (first 32 ents), k, v -> 8 heads x 64
    logits = q k^T / 8, masked by pre_mask (True = masked out)
    w = softmax(logits), fully-masked rows -> 0
    out = (w v) @ W_out.T + b_out, zeroed where post_mask

Sharding: data-parallel over batch, 128 batches per core.

Per-core kernel design (bf16 matmuls; fp8 fails the accuracy budget):
  - QKV computed feature-major (q^T, k^T: [feat, tok]) feeding the logits
    matmuls directly; V token-major ([ent, feat]) feeding attn@v.
  - logits for all 8 heads of a batch run as one 8-slot tile_position group;
    the two PE row-halves write separate PSUM banks (sharing one bank between
    row tiles is a hardware fault).
  - softmax over the free (ent) axis, fused: exp (scalar, bf16 out) ->
    tensor_tensor_reduce (mask-multiply + row-sum + 1e-30 init in one DVE
    pass) -> reciprocal_approx_fast -> in-place per-partition scale.
    Fully-masked rows stay finite and produce exact zeros like the reference.
  - w is PE-transposed per (batch, head-parity) so attn@v contracts over
    entities; attn lands feature-major, feeding the out-projection directly;
    output is stored [out_feat, batch, agent] and untransposed on the host.
  - bias-add + post-mask fused into one scalar_tensor_tensor per mo chunk.
  - The attention work of iteration N-1 is split into 8 fine-grained units
    (logits+softmax x2, transpose x2, attn@v x2, out-proj x2) interleaved
    between iteration N's 20 dense QKV units, so every cross-engine
    dependency (PE -> Act -> DVE -> PE) has several microseconds of dense
    matmul work in front of it and the PE never stalls on the softmax chain.
  - PSUM drains are spread over Act and DVE (GPSIMD has no PSUM port).
"""
import sys

sys.path.insert(0, "/opt/trn_rl_repo")

import numpy as np
import ml_dtypes

BS, NE, IN_DIM = 1024, 128, 512
EMBED, OUT_DIM = 512, 512
N_HEADS, N_AGENTS = 8, 32
HEAD_DIM = EMBED // N_HEADS  # 64
N_CORES = 8


def build_nc(b_core: int):
    """Build the per-core Bass program for b_core batches (b_core % 8 == 0)."""
    import concourse.bass as bass
    import concourse.tile as tile
    from concourse import bacc, mybir
    from concourse.masks import make_identity

    F32 = mybir.dt.float32
    BF16 = mybir.dt.bfloat16
    Exp = mybir.ActivationFunctionType.Exp
    Mult = mybir.AluOpType.mult
    Add = mybir.AluOpType.add

    assert b_core % 8 == 0
    n_iter = b_core // 8

    nc = bacc.Bacc("TRN2", target_bir_lowering=False, debug=False)

    xt_d = nc.declare_dram_parameter("xt", [b_core, IN_DIM, NE], BF16, isOutput=False)
    xta_d = nc.declare_dram_parameter("xta", [4, 128, b_core, N_AGENTS], BF16, isOutput=False)
    wi_d = nc.declare_dram_parameter("wi", [IN_DIM, 3 * EMBED], BF16, isOutput=False)
    wo_d = nc.declare_dram_parameter("wo", [EMBED, OUT_DIM], BF16, isOutput=False)
    keep_d = nc.declare_dram_parameter("keep", [b_core, N_AGENTS, NE], BF16, isOutput=False)
    pkeep_d = nc.declare_dram_parameter("pkeep", [b_core, N_AGENTS], F32, isOutput=False)
    bias_d = nc.declare_dram_parameter("bias", [OUT_DIM], F32, isOutput=False)
    out_d = nc.declare_dram_parameter("out", [OUT_DIM, b_core, N_AGENTS], F32, isOutput=True)

    AP = bass.AP

    def dram_ap(handle, offset, ap):
        base = handle[:]
        return AP(tensor=base.tensor, offset=offset, ap=ap)

    with tile.TileContext(nc) as tc:
        with (
            tc.tile_pool(name="const", bufs=1) as constp,
            tc.tile_pool(name="ins", bufs=2) as insp,
            tc.tile_pool(name="mid", bufs=2) as midp,
            tc.tile_pool(name="attn", bufs=2) as attnp,
            tc.tile_pool(name="outs", bufs=2) as outsp,
            tc.tile_pool(name="ps_mm", bufs=3, space="PSUM") as ps_mm,
            tc.tile_pool(name="ps_lg", bufs=1, space="PSUM") as ps_lg,
            tc.tile_pool(name="ps_wt", bufs=1, space="PSUM") as ps_wt,
            tc.tile_pool(name="ps_at", bufs=1, space="PSUM") as ps_at,
            tc.tile_pool(name="ps_op", bufs=1, space="PSUM") as ps_op,
        ):
            # ---- constants ----
            # W_in loaded in three column groups (q cols first so the first
            # iteration's q matmuls can start before the k/v columns land).
            wi_sb = [
                constp.tile([128, 3 * EMBED], BF16, name=f"wi_{ki}", tag=f"wi_{ki}")
                for ki in range(4)
            ]

            def emit_wi_cols(c0, width):
                for ki in range(4):
                    nc.sync.dma_start(
                        out=wi_sb[ki][:, c0 : c0 + width],
                        in_=dram_ap(
                            wi_d,
                            ki * 128 * 3 * EMBED + c0,
                            [[3 * EMBED, 128], [1, width]],
                        ),
                    )

            wo_sb = constp.tile([128, 4, OUT_DIM], BF16)
            bias_sb = constp.tile([128, 4], F32)
            ident = constp.tile([128, 128], BF16)

            def emit_late_consts():
                nc.scalar.dma_start(
                    out=wo_sb,
                    in_=dram_ap(wo_d, 0, [[OUT_DIM, 128], [128 * OUT_DIM, 4], [1, OUT_DIM]]),
                )
                nc.scalar.dma_start(out=bias_sb, in_=dram_ap(bias_d, 0, [[1, 128], [128, 4]]))
                make_identity(nc, ident)

            def emit_inputs(it):
                """Issue this iter's input DMAs; returns the state dict."""
                b0 = it * 8
                st = {"it": it}
                st["xta"] = xta_sb = insp.tile(
                    [128, 4, 8, N_AGENTS], BF16, name="xta_sb", tag="xta_sb"
                )
                for ki in range(4):
                    nc.sync.dma_start(
                        out=xta_sb[:, ki, :, :],
                        in_=dram_ap(
                            xta_d,
                            ki * 128 * b_core * N_AGENTS + b0 * N_AGENTS,
                            [[b_core * N_AGENTS, 128], [N_AGENTS, 8], [1, N_AGENTS]],
                        ),
                    )
                # keep mask, replicated over the 4 head-pair partition groups
                st["keep"] = keep_bc = insp.tile(
                    [128, 8, NE], BF16, name="keep_bc", tag="keep_bc"
                )
                for cg in range(4):
                    nc.gpsimd.dma_start(
                        out=keep_bc[cg * 32 : (cg + 1) * 32, :, :],
                        in_=dram_ap(
                            keep_d,
                            b0 * N_AGENTS * NE,
                            [[NE, 32], [N_AGENTS * NE, 8], [1, NE]],
                        ),
                    )
                st["pkeep"] = pkeep_bc = insp.tile(
                    [128, 8, N_AGENTS], F32, name="pkeep_bc", tag="pkeep_bc"
                )
                nc.gpsimd.dma_start(
                    out=pkeep_bc,
                    in_=dram_ap(pkeep_d, b0 * N_AGENTS, [[0, 128], [N_AGENTS, 8], [1, N_AGENTS]]),
                )
                st["xt"] = xt_sb = insp.tile([128, 4, 8, NE], BF16, name="xt_sb", tag="xt_sb")
                for ki in range(4):
                    nc.sync.dma_start(
                        out=xt_sb[:, ki, :, :],
                        in_=dram_ap(
                            xt_d,
                            b0 * IN_DIM * NE + ki * 128 * NE,
                            [[NE, 128], [IN_DIM * NE, 8], [1, NE]],
                        ),
                    )
                st["qt"] = midp.tile([128, 4, 8, N_AGENTS], BF16, name="qt_sb", tag="qt_sb")
                st["kt"] = midp.tile([128, 4, 8, NE], BF16, name="kt_sb", tag="kt_sb")
                st["vt"] = midp.tile([128, 8, EMBED], BF16, name="vt_sb", tag="vt_sb")
                return st

            def emit_q_unit(st, mo):
                q_ps = ps_mm.tile([128, 8, N_AGENTS], F32, tag="mm", name="q_ps")
                for ki in range(4):
                    nc.tensor.matmul(
                        q_ps,
                        wi_sb[ki][:, mo * 128 : (mo + 1) * 128],
                        st["xta"][:, ki, :, :],
                        start=(ki == 0),
                        stop=(ki == 3),
                    )
                nc.vector.tensor_copy(out=st["qt"][:, mo, :, :], in_=q_ps)

            def emit_k_unit(st, mo, g2):
                k_ps = ps_mm.tile([128, 4, NE], F32, tag="mm", name="k_ps")
                for ki in range(4):
                    nc.tensor.matmul(
                        k_ps,
                        wi_sb[ki][:, EMBED + mo * 128 : EMBED + (mo + 1) * 128],
                        st["xt"][:, ki, g2 * 4 : (g2 + 1) * 4, :],
                        start=(ki == 0),
                        stop=(ki == 3),
                    )
                nc.scalar.copy(out=st["kt"][:, mo, g2 * 4 : (g2 + 1) * 4, :], in_=k_ps)

            def emit_v_unit(st, b):
                v_ps = ps_mm.tile([128, EMBED], F32, tag="mm", name="v_ps")
                for ki in range(4):
                    nc.tensor.matmul(
                        v_ps,
                        st["xt"][:, ki, b, :],
                        wi_sb[ki][:, 2 * EMBED : 3 * EMBED],
                        start=(ki == 0),
                        stop=(ki == 3),
                    )
                # v6/v7 drain on the (by then idle) Act engine so the next
                # iteration's q_ps allocations in the shared PSUM ring don't
                # wait on the vector queue tail
                if b % 2 == 0 or b >= 6:
                    nc.scalar.copy(out=st["vt"][:, b, :], in_=v_ps)
                else:
                    nc.vector.tensor_copy(out=st["vt"][:, b, :], in_=v_ps)

            def emit_attn_A(st, sc):
                """logits for 4 batches + fused softmax -> st["wn{sc}"]."""
                qt_sb, kt_sb = st["qt"], st["kt"]
                # separate psum tiles per row-half (shared bank = HW fault)
                lg = [
                    ps_lg.tile([128, 4, NE], F32, tag="lg0", name="lg0"),
                    ps_lg.tile([128, 4, NE], F32, tag="lg1", name="lg1"),
                ]  # [(h//2)*32+a, bs, e] for h%2 = 0, 1
                for bs in range(4):
                    b = sc * 4 + bs
                    for h in range(8):
                        rh, cg = h % 2, h // 2
                        nc.tensor.matmul(
                            lg[rh][cg * 32 : (cg + 1) * 32, bs, :],
                            qt_sb[rh * 64 : rh * 64 + 64, cg, b, :],
                            kt_sb[rh * 64 : rh * 64 + 64, cg, b, :],
                            start=True,
                            stop=True,
                            tile_position=(rh * 64, cg * 32),
                        )
                we = attnp.tile([128, 4, 2, NE], BF16, name="we", tag="we")
                for rh in range(2):
                    nc.scalar.activation(out=we[:, :, rh, :], in_=lg[rh], func=Exp, scale=0.125)
                sums = attnp.tile([128, 4, 2], F32, name="sums", tag="sums")
                wn = attnp.tile([128, 4, 2, NE], BF16, name="wn", tag="wn")
                # mask-multiply all 8 (bs, rh) rows in one DVE pass (keep is
                # broadcast over the rh axis with a stride-0 AP)
                keep_bc4 = (
                    st["keep"][:, sc * 4 : (sc + 1) * 4, :]
                    .unsqueeze(2)
                    .broadcast_to([128, 4, 2, NE])
                )
                nc.vector.tensor_mul(wn, we, keep_bc4)
                nc.vector.reduce_sum(sums, wn, axis=mybir.AxisListType.X)
                nc.vector.tensor_scalar_add(sums, sums, 1e-30)
                rcp = attnp.tile([128, 4, 2], F32, name="rcp", tag="rcp")
                nc.vector.reciprocal_approx_fast(out=rcp, in_=sums)
                # normalize in one pass (rcp broadcast along the entity axis)
                nc.vector.tensor_mul(
                    wn, wn, rcp.unsqueeze(3).broadcast_to([128, 4, 2, NE])
                )
                st[f"wn{sc}"] = wn

            def emit_attn_Bt(st, sc):
                """PE-transpose the normalized weights: wn -> wt (ent-major)."""
                wn = st[f"wn{sc}"]
                wt_ps = ps_wt.tile([128, 4, 2, NE], BF16, name="wt_ps")
                for bs in range(4):
                    for rh in range(2):
                        nc.tensor.transpose(wt_ps[:, bs, rh, :], wn[:, bs, rh, :], ident)
                wt_sb = attnp.tile([128, 4, 2, NE], BF16, name="wt_sb", tag="wt_sb")
                nc.vector.tensor_copy(out=wt_sb, in_=wt_ps)
                st[f"wt{sc}"] = wt_sb

            def emit_attn_Bv(st, sc):
                """attn = w @ v, feature-major, then drain to SBUF."""
                vt_sb, wt_sb = st["vt"], st[f"wt{sc}"]
                at_ps = ps_at.tile([128, 4, 4, N_AGENTS], F32, name="at_ps", tag="at")
                for bs in range(4):
                    b = sc * 4 + bs
                    for h in range(8):
                        rh, cg = h % 2, h // 2
                        nc.tensor.matmul(
                            at_ps[rh * 64 : rh * 64 + 64, bs, cg, :],
                            vt_sb[:, b, h * 64 : (h + 1) * 64],
                            wt_sb[:, bs, rh, cg * 32 : (cg + 1) * 32],
                            start=True,
                            stop=True,
                            tile_position=(0, rh * 64),
                        )
                attn_sb = outsp.tile([128, 4, 4, N_AGENTS], BF16, name="attn_sb", tag="attn_sb")
                nc.scalar.copy(out=attn_sb, in_=at_ps)
                st[f"attn{sc}"] = attn_sb

            def emit_outproj(st, sc):
                b0 = st["it"] * 8 + sc * 4
                attn_sb = st[f"attn{sc}"]
                op_ps = ps_op.tile([128, 4, 4, N_AGENTS], F32, name="op_ps")
                for mo in range(4):
                    for ki2 in range(4):
                        nc.tensor.matmul(
                            op_ps[:, mo, :, :],
                            wo_sb[:, ki2, mo * 128 : (mo + 1) * 128],
                            attn_sb[:, :, ki2, :],
                            start=(ki2 == 0),
                            stop=(ki2 == 3),
                        )
                out_sb = outsp.tile([128, 4, 4, N_AGENTS], F32, name="out_sb", tag="out_sb")
                bias_bc = (
                    bias_sb[:, :].unsqueeze(2).unsqueeze(3)
                    .broadcast_to([128, 4, 4, N_AGENTS])
                )
                pkeep_bc4 = (
                    st["pkeep"][:, sc * 4 : (sc + 1) * 4, :]
                    .unsqueeze(1)
                    .broadcast_to([128, 4, 4, N_AGENTS])
                )
                nc.vector.tensor_add(out_sb, op_ps, bias_bc)
                nc.vector.tensor_mul(out_sb, out_sb, pkeep_bc4)
                nc.scalar.dma_start(
                    out=dram_ap(
                        out_d,
                        b0 * N_AGENTS,
                        [[b_core * N_AGENTS, 128],
                         [128 * b_core * N_AGENTS, 4],
                         [N_AGENTS, 4],
                         [1, N_AGENTS]],
                    ),
                    in_=out_sb,
                )

            def qkv_units(st):
                units = []
                for mo in range(4):
                    units.append(lambda mo=mo: emit_q_unit(st, mo))
                for mo in range(4):
                    for g2 in range(2):
                        units.append(lambda mo=mo, g2=g2: emit_k_unit(st, mo, g2))
                for b in range(8):
                    units.append(lambda b=b: emit_v_unit(st, b))
                return units

            def attn_units(st):
                return [
                    lambda: emit_attn_A(st, 0),
                    lambda: emit_attn_A(st, 1),
                    lambda: emit_attn_Bt(st, 0),
                    lambda: emit_attn_Bv(st, 0),
                    lambda: emit_outproj(st, 0),
                    lambda: emit_attn_Bt(st, 1),
                    lambda: emit_attn_Bv(st, 1),
                    lambda: emit_outproj(st, 1),
                ]

            # qu indices before which the next attention unit is inserted
            AU_SLOTS = (0, 5, 8, 10, 12, 14, 16, 18)

            # software pipeline: interleave iter N's QKV with iter N-1's
            # attention, at fine granularity
            emit_wi_cols(0, EMBED)  # q columns first
            st0 = emit_inputs(0)
            emit_wi_cols(EMBED, EMBED)      # k columns
            emit_wi_cols(2 * EMBED, EMBED)  # v columns
            emit_late_consts()
            prev = None
            for it in range(n_iter):
                st = st0 if it == 0 else emit_inputs(it)
                qu = qkv_units(st)
                au = attn_units(prev) if prev is not None else []
                ai = 0
                for i, u in enumerate(qu):
                    if au and ai < len(au) and i in AU_SLOTS:
                        au[ai]()
                        ai += 1
                    u()
                while ai < len(au):
                    au[ai]()
                    ai += 1
                prev = st
            for u in attn_units(prev):
                u()

    nc.compile()
    return nc


def _prep_core_inputs(ents, keep, pkeep, wi, wo, bias):
    """Host-side layout prep for one core's batch shard."""
    b_core = ents.shape[0]
    xt = np.ascontiguousarray(ents.transpose(0, 2, 1))  # [b, in, e]
    xta = np.ascontiguousarray(
        ents[:, :N_AGENTS, :].transpose(2, 0, 1)
    ).reshape(4, 128, b_core, N_AGENTS)
    return {
        "xt": xt,
        "xta": xta,
        "wi": wi,
        "wo": wo,
        "keep": keep,
        "pkeep": pkeep,
        "bias": bias,
    }


def run(entities, pre_mask, post_mask, W_in, W_out, b_out, trace=False):
    """Shard, run on 8 cores, gather. Returns (out, BassKernelResults)."""
    from concourse.bass_utils import run_bass_kernel_spmd

    bs = entities.shape[0]
    b_core = bs // N_CORES
    entities = np.asarray(entities, dtype=np.float32).astype(ml_dtypes.bfloat16)
    keep = (~np.asarray(pre_mask)).astype(ml_dtypes.bfloat16)
    pkeep = (~np.asarray(post_mask)).astype(np.float32)
    wi = np.ascontiguousarray(np.asarray(W_in, dtype=np.float32).T).astype(ml_dtypes.bfloat16)
    wo = np.ascontiguousarray(np.asarray(W_out, dtype=np.float32).T).astype(ml_dtypes.bfloat16)
    bias = np.asarray(b_out, dtype=np.float32)

    nc = build_nc(b_core)
    in_maps = [
        _prep_core_inputs(
            entities[c * b_core : (c + 1) * b_core],
            keep[c * b_core : (c + 1) * b_core],
            pkeep[c * b_core : (c + 1) * b_core],
            wi, wo, bias,
        )
        for c in range(N_CORES)
    ]
    res = run_bass_kernel_spmd(nc, in_maps, list(range(N_CORES)), trace=trace)
    out = np.empty((bs, N_AGENTS, OUT_DIM), dtype=np.float32)
    for c in range(N_CORES):
        out[c * b_core : (c + 1) * b_core] = res.results[c]["out"].transpose(1, 2, 0)
    return out, res


def kernel(entities, pre_mask, post_mask, W_in, W_out, b_out):
    out, _ = run(entities, pre_mask, post_mask, W_in, W_out, b_out, trace=False)
    return out
